# revision 21
# baseline (speedup 1.0000x reference)
"""Multi-head attention (B=4, L=2048, D=1024, H=16, dk=dv=64) on 8 trn2 cores.

Sharding: core c = b*2 + half; each core does batch b, 8 heads.
All attention math in the transposed (S^T) orientation; attn is written to
DRAM in a flat tile-order layout and the host reassembles [B,H,L,L].

  qhT/khT [d, l] = Wx_half @ x[b]^T          (fp32r matmuls, D-chunked)
  S^T[k,q] = sum_d khT[d,k] qhT[d,q]         (K=64, head pair row-packed)
  E^T = exp(S^T/8)                           (ScalarE, PSUM->SBUF, fp32r)
  PV: [vh | 1]^T-mm E^T -> O^T[dv,q], denom[q] in row 64 (PSUM fp32)
  recip = exp(-ln(denom))  (ScalarE)  -> replicated via GpSimd broadcast
  attn^T = E^T * recip     (DVE, staged 4-chunks -> contiguous DMA)
  ctx^T = O^T * recip      -> DRAM; out_p = ctx^T-mm Wo^T per q-block
"""

import numpy as np

import concourse.bacc as bacc
import concourse.mybir as mybir
import concourse.tile as tile
from concourse import bass
from concourse.bass_utils import run_bass_kernel_spmd

B, L, D = 4, 2048, 1024
H, DK, DV = 16, 64, 64
HPC = 8  # heads per core
NCORES = 8
QBLK = 256
NQB = L // QBLK  # 8
NDC = D // 128  # 8
NLT = L // 128  # 16
NPAIR = HPC // 2

F32 = mybir.dt.float32
F32R = mybir.dt.float32r  # ~1.5e-4 rel rounding, full PE speed at N>=256
AF = mybir.ActivationFunctionType

# Exp defaults to "exp_and_others" and Ln to "natural_log", which makes the
# per-head Exp/Ln alternation reload ACT tables (~1.3us each, 63x). Restrict
# the chooser to sets that either hold both or hold neither.
from concourse.hw_specs import get_activation_tables as _gat


def _gat_shared(arch):
    # Index into this dict is the act_func_set_id consumed by walrus, so
    # entries must stay in place — only strip Exp/Ln from the other sets.
    out = {}
    for name, fns in _gat(arch).items():
        if name != "natural_log_exp_and_others" and ({AF.Exp, AF.Ln} & fns):
            fns = fns - {AF.Exp, AF.Ln}
        out[name] = fns
    return out


bacc.get_activation_tables = _gat_shared


def build_program():
    nc = bacc.Bacc(trn_type="TRN2", name="mha_spmd")
    qT = nc.dram_tensor("qT", [D, L], F32R, kind="ExternalInput")
    kT = nc.dram_tensor("kT", [D, L], F32R, kind="ExternalInput")
    vT = nc.dram_tensor("vT", [D, L], F32R, kind="ExternalInput")
    wq = nc.dram_tensor("wq", [D, HPC * DK], F32R, kind="ExternalInput")
    wk = nc.dram_tensor("wk", [D, HPC * DK], F32R, kind="ExternalInput")
    wv = nc.dram_tensor("wv", [D, HPC * DV], F32R, kind="ExternalInput")
    wo = nc.dram_tensor("wo", [HPC * DV, D], F32R, kind="ExternalInput")
    onesc = nc.dram_tensor("onesc", [128, HPC], F32R, kind="ExternalInput")
    # attn in flat tile order: [head, qb, p, t, q] ; host reassembles
    attn_f = nc.dram_tensor(
        "attn_f", [HPC, NQB, 128, NLT, QBLK], F32R, kind="ExternalOutput"
    )
    out_p = nc.dram_tensor("out_p", [L, D], F32, kind="ExternalOutput")
    # on-device scratch (qh and ctx stream through DRAM to fit SBUF)
    qh_d = nc.dram_tensor("qh_d", [NPAIR, NQB // 2, 128, 2 * QBLK], F32R)
    ctx_d = nc.dram_tensor("ctx_d", [NQB, NPAIR, 2, 64, QBLK], F32R)

    with tile.TileContext(nc) as tc:
        with tc.tile_pool(name="persist", bufs=1) as pp:
            khT = [
                pp.tile([128, L], F32R, tag=f"khT{p}", name=f"khT{p}")
                for p in range(NPAIR)
            ]
            vho = [
                pp.tile([128, HPC * (DV + 1)], F32R, tag=f"vho{t}", name=f"vho{t}")
                for t in range(NLT)
            ]

            # ---------------- stage A: projections ----------------
            with (
                tc.tile_pool(name="wgt", bufs=1) as wp,
                tc.tile_pool(name="stream", bufs=2) as sp,
                tc.tile_pool(name="qstage", bufs=2) as qsp,
                tc.tile_pool(name="apsum", bufs=4, space="PSUM") as appool,
            ):
                # --- q projection -> qh_d (DRAM) ---
                wq_sb = []
                for i in range(NDC):
                    wt = wp.tile([128, HPC * DK], F32R, tag=f"wq{i}", name=f"wq{i}")
                    nc.sync.dma_start(wt[:], wq[i * 128 : (i + 1) * 128, :])
                    wq_sb.append(wt)
                for n in range(NQB // 2):
                    xs = []
                    for kk in range(NDC):
                        xt = sp.tile([128, 512], F32R, tag=f"x{kk}", name=f"x{kk}")
                        nc.sync.dma_start(
                            xt[:],
                            qT[kk * 128 : (kk + 1) * 128, n * 512 : (n + 1) * 512],
                        )
                        xs.append(xt)
                    for m in range(NPAIR):
                        ps = appool.tile([128, 512], F32, tag="projps", name="projps")
                        for kk in range(NDC):
                            nc.tensor.matmul(
                                ps[:],
                                lhsT=wq_sb[kk][:, m * 128 : (m + 1) * 128],
                                rhs=xs[kk][:],
                                start=(kk == 0),
                                stop=(kk == NDC - 1),
                            )
                        qst = qsp.tile([128, 512], F32R, tag="qst", name="qst")
                        nc.vector.tensor_copy(qst[:], ps[:])
                        nc.sync.dma_start(qh_d[m, n], qst[:])

                # --- k projection -> khT (SBUF resident) ---
                wk_sb = []
                for i in range(NDC):
                    wt = wp.tile([128, HPC * DK], F32R, tag=f"wk{i}", name=f"wk{i}")
                    nc.sync.dma_start(wt[:], wk[i * 128 : (i + 1) * 128, :])
                    wk_sb.append(wt)
                for n in range(NQB // 2):
                    xs = []
                    for kk in range(NDC):
                        xt = sp.tile([128, 512], F32R, tag=f"x{kk}", name=f"x{kk}")
                        nc.sync.dma_start(
                            xt[:],
                            kT[kk * 128 : (kk + 1) * 128, n * 512 : (n + 1) * 512],
                        )
                        xs.append(xt)
                    for m in range(NPAIR):
                        ps = appool.tile([128, 512], F32, tag="projps", name="projps")
                        for kk in range(NDC):
                            nc.tensor.matmul(
                                ps[:],
                                lhsT=wk_sb[kk][:, m * 128 : (m + 1) * 128],
                                rhs=xs[kk][:],
                                start=(kk == 0),
                                stop=(kk == NDC - 1),
                            )
                        nc.vector.tensor_copy(
                            khT[m][:, n * 512 : (n + 1) * 512], ps[:]
                        )

                # --- v projection -> vho (SBUF resident, with ones col) ---
                wv_sb = []
                for i in range(NDC):
                    wt = wp.tile([128, HPC * DV], F32R, tag=f"wv{i}", name=f"wv{i}")
                    nc.sync.dma_start(wt[:], wv[i * 128 : (i + 1) * 128, :])
                    wv_sb.append(wt)
                for tg in range(4):
                    xs = []
                    for kk in range(NDC):
                        xt = sp.tile([128, 512], F32R, tag=f"x{kk}", name=f"x{kk}")
                        nc.sync.dma_start(
                            xt[:],
                            vT[kk * 128 : (kk + 1) * 128, tg * 512 : (tg + 1) * 512],
                        )
                        xs.append(xt)
                    for ti in range(4):
                        t = tg * 4 + ti
                        ps = appool.tile([128, 512], F32, tag="projps", name="projps")
                        for kk in range(NDC):
                            nc.tensor.matmul(
                                ps[:],
                                lhsT=xs[kk][:, ti * 128 : (ti + 1) * 128],
                                rhs=wv_sb[kk][:],
                                start=(kk == 0),
                                stop=(kk == NDC - 1),
                            )
                        dst3 = vho[t][:].rearrange("p (h x) -> p h x", x=DV + 1)
                        src3 = ps[:].rearrange("p (h x) -> p h x", x=DV)
                        nc.vector.tensor_copy(dst3[:, :, 0:DV], src3)
                        nc.sync.dma_start(
                            dst3[:, :, DV : DV + 1],
                            onesc[:].rearrange("p (h x) -> p h x", x=1),
                        )

            # ---------------- stage B: attention per (qb, pair) ----------
            with (
                tc.tile_pool(name="wopool", bufs=1) as wop,
                tc.tile_pool(name="epool", bufs=2) as ep,
                tc.tile_pool(name="qread", bufs=2) as qrp,
                tc.tile_pool(name="cstage", bufs=2) as csp,
                tc.tile_pool(name="cread", bufs=1) as crp,
                tc.tile_pool(name="astage", bufs=2) as asp,
                tc.tile_pool(name="small", bufs=2) as smp,
                tc.tile_pool(name="outsb", bufs=1) as osb,
                tc.tile_pool(name="spsum", bufs=2, space="PSUM") as spp,
                tc.tile_pool(name="vpsum", bufs=1, space="PSUM") as vpp,
                tc.tile_pool(name="opsum", bufs=2, space="PSUM") as opp,
            ):
                wo_sb = []
                for i in range(NPAIR):
                    wt = wop.tile([128, D], F32R, tag=f"wo{i}", name=f"wo{i}")
                    nc.sync.dma_start(wt[:], wo[i * 128 : (i + 1) * 128, :])
                    wo_sb.append(wt)

                for qb in range(NQB):
                    for p in range(NPAIR):
                        qhr = qrp.tile([128, QBLK], F32R, tag="qhr", name="qhr")
                        nc.sync.dma_start(
                            qhr[:],
                            qh_d[p, qb // 2][:, (qb % 2) * QBLK : (qb % 2 + 1) * QBLK],
                        )
                        # epair layout per 1024-col group g (t = 2g + t2):
                        #   [A(2g) | A(2g+1) | B(2g) | B(2g+1)]  (256 each)
                        epair = ep.tile(
                            [128, NLT * 2 * QBLK], F32R, tag="epair", name="epair"
                        )
                        pos = [
                            vpp.tile([128, QBLK], F32, tag=f"pv{jj}", name=f"pv{jj}")
                            for jj in range(2)
                        ]
                        for g in range(NLT // 2):
                            sps = spp.tile(
                                [128, 4 * QBLK], F32, tag="score", name="score"
                            )
                            for t2 in range(2):
                                t = 2 * g + t2
                                for jj in range(2):
                                    base = jj * 64
                                    nc.tensor.matmul(
                                        sps[
                                            :,
                                            (jj * 2 + t2) * QBLK : (jj * 2 + t2 + 1)
                                            * QBLK,
                                        ],
                                        lhsT=khT[p][
                                            base : base + 64, t * 128 : (t + 1) * 128
                                        ],
                                        rhs=qhr[base : base + 64, :],
                                        start=True,
                                        stop=True,
                                        tile_position=(base, 0),
                                    )
                            nc.scalar.activation(
                                epair[:, g * 4 * QBLK : (g + 1) * 4 * QBLK],
                                sps[:],
                                AF.Exp,
                                scale=0.125,
                            )
                            for t2 in range(2):
                                t = 2 * g + t2
                                for jj in range(2):
                                    j = p * 2 + jj
                                    nc.tensor.matmul(
                                        pos[jj][0 : DV + 1, :],
                                        lhsT=vho[t][
                                            :, j * (DV + 1) : (j + 1) * (DV + 1)
                                        ],
                                        rhs=epair[
                                            :,
                                            (g * 4 + jj * 2 + t2) * QBLK : (
                                                g * 4 + jj * 2 + t2 + 1
                                            )
                                            * QBLK,
                                        ],
                                        start=(t == 0),
                                        stop=(t == NLT - 1),
                                    )
                        # e5: [p][g 8][jj 2][t2 2][q]
                        e5 = epair[:].rearrange(
                            "p (g jj t2 q) -> p g jj t2 q", jj=2, t2=2, q=QBLK
                        )
                        for jj in range(2):
                            j = p * 2 + jj
                            po = pos[jj]
                            lnd = smp.tile([1, QBLK], F32, tag="lnd", name="lnd")
                            nc.scalar.activation(lnd[:], po[DV : DV + 1, :], AF.Ln)
                            rec = smp.tile([1, QBLK], F32, tag="rec", name="rec")
                            nc.scalar.activation(rec[:], lnd[:], AF.Exp, scale=-1.0)
                            rep = smp.tile([128, QBLK], F32, tag="rep", name="rep")
                            nc.gpsimd.partition_broadcast(rep[:], rec[:])
                            # normalized context -> DRAM (GpSimd: small op)
                            cst = csp.tile([64, QBLK], F32R, tag="cst", name="cst")
                            nc.vector.tensor_mul(cst[:], po[0:DV, :], rep[0:DV, :])
                            nc.sync.dma_start(ctx_d[qb, p, jj], cst[:])
                            # normalize E^T (one 4096-wide op), stage, DMA out
                            ast = asp.tile(
                                [128, 16 * QBLK], F32R, tag="ast", name="ast"
                            )
                            e3 = e5[:, :, jj, :, :]
                            a3 = ast[:].rearrange(
                                "p (g t2 q) -> p g t2 q", t2=2, q=QBLK
                            )
                            r4 = rep[:].rearrange(
                                "p (g t2 q) -> p g t2 q", g=1, t2=1
                            )
                            r4b, _ = bass.broadcast_tensor_aps(r4, e3)
                            nc.vector.tensor_mul(a3, e3, r4b)
                            nc.sync.dma_start(attn_f[j, qb], a3)

                    # ---- output projection for this q-block ----
                    ctx_sb = []
                    for kk in range(NPAIR):
                        ct = crp.tile(
                            [128, QBLK], F32R, tag=f"ctr{kk}", name=f"ctr{kk}"
                        )
                        nc.sync.dma_start(
                            ct[:],
                            ctx_d[qb, kk].rearrange("a b c -> (a b) c"),
                        )
                        ctx_sb.append(ct)
                    for mq in range(QBLK // 128):
                        row = slice(qb * QBLK + mq * 128, qb * QBLK + (mq + 1) * 128)
                        ob = osb.tile([128, D], F32, tag="ob", name="ob")
                        for dc in range(2):
                            ps = opp.tile([128, 512], F32, tag="ops", name="ops")
                            for kk in range(NPAIR):
                                nc.tensor.matmul(
                                    ps[:],
                                    lhsT=ctx_sb[kk][:, mq * 128 : (mq + 1) * 128],
                                    rhs=wo_sb[kk][:, dc * 512 : (dc + 1) * 512],
                                    start=(kk == 0),
                                    stop=(kk == NPAIR - 1),
                                )
                            nc.vector.tensor_copy(
                                ob[:, dc * 512 : (dc + 1) * 512], ps[:]
                            )
                        nc.sync.dma_start(out_p[row, :], ob[:])

    nc.finalize()
    return nc


_PROGRAM_CACHE = {}


def _get_program():
    if "nc" not in _PROGRAM_CACHE:
        _PROGRAM_CACHE["nc"] = build_program()
    return _PROGRAM_CACHE["nc"]


def _shard_inputs(q, k, v, Wq, Wk, Wv, Wo):
    WoT = np.ascontiguousarray(Wo.T.astype(np.float32))
    in_maps = []
    for c in range(NCORES):
        b, hf = divmod(c, 2)
        rows = slice(hf * HPC * DK, (hf + 1) * HPC * DK)
        in_maps.append(
            {
                "qT": np.ascontiguousarray(q[b].T.astype(np.float32)),
                "kT": np.ascontiguousarray(k[b].T.astype(np.float32)),
                "vT": np.ascontiguousarray(v[b].T.astype(np.float32)),
                "wq": np.ascontiguousarray(Wq[rows].T.astype(np.float32)),
                "wk": np.ascontiguousarray(Wk[rows].T.astype(np.float32)),
                "wv": np.ascontiguousarray(Wv[rows].T.astype(np.float32)),
                "wo": np.ascontiguousarray(WoT[rows]),
                "onesc": np.ones((128, HPC), np.float32),
            }
        )
    return in_maps


def run(q, k, v, Wq, Wk, Wv, Wo, trace=False, trace_kwargs=None):
    nc = _get_program()
    in_maps = _shard_inputs(q, k, v, Wq, Wk, Wv, Wo)
    res = run_bass_kernel_spmd(
        nc,
        in_maps,
        core_ids=list(range(NCORES)),
        trace=trace,
        **(trace_kwargs or {}),
    )
    out = np.empty((B, L, D), np.float32)
    attn = np.empty((B, H, L, L), np.float32)
    for c in range(NCORES):
        b, hf = divmod(c, 2)
        r = res.results[c]
        if hf == 0:
            out[b] = r["out_p"]
        else:
            out[b] += r["out_p"]
        # attn_f [head, qb, p, t, q]: attn[q_g, k_g] with q_g = qb*QBLK+q,
        # k_g = t*128 + p  ->  transpose to [head, qb, q, t, p]
        af = r["attn_f"]
        attn[b, hf * HPC : (hf + 1) * HPC] = af.transpose(0, 1, 4, 3, 2).reshape(
            HPC, L, L
        )
    return out, attn, res


def kernel(q, k, v, Wq, Wk, Wv, Wo):
    out, attn, _ = run(q, k, v, Wq, Wk, Wv, Wo)
    return out, attn


# revision 23
# speedup vs baseline: 1.0928x; 1.0928x over previous
"""Multi-head attention (B=4, L=2048, D=1024, H=16, dk=dv=64) on 8 trn2 cores.

Sharding: core c = b*2 + half; each core does batch b, 8 heads.
All attention math in the transposed (S^T) orientation; attn is written to
DRAM in a flat tile-order layout and the host reassembles [B,H,L,L].

  qhT/khT [d, l] = Wx_half @ x[b]^T          (fp32r matmuls, D-chunked)
  S^T[k,q] = sum_d khT[d,k] qhT[d,q]         (K=64, head pair row-packed)
  E^T = exp(S^T/8)                           (ScalarE, PSUM->SBUF, fp32r)
  PV: [vh | 1]^T-mm E^T -> O^T[dv,q], denom[q] in row 64 (PSUM fp32)
  recip = exp(-ln(denom))  (ScalarE)  -> replicated via GpSimd broadcast
  attn^T = E^T * recip     (DVE, staged 4-chunks -> contiguous DMA)
  ctx^T = O^T * recip      -> DRAM; out_p = ctx^T-mm Wo^T per q-block
"""

import numpy as np

import concourse.bacc as bacc
import concourse.mybir as mybir
import concourse.tile as tile
from concourse import bass
from concourse.bass_utils import run_bass_kernel_spmd

B, L, D = 4, 2048, 1024
H, DK, DV = 16, 64, 64
HPC = 8  # heads per core
NCORES = 8
QBLK = 256
NQB = L // QBLK  # 8
NDC = D // 128  # 8
NLT = L // 128  # 16
NPAIR = HPC // 2

F32 = mybir.dt.float32
F32R = mybir.dt.float32r  # ~1.5e-4 rel rounding, full PE speed at N>=256
AF = mybir.ActivationFunctionType

# Exp defaults to "exp_and_others" and Ln to "natural_log", which makes the
# per-head Exp/Ln alternation reload ACT tables (~1.3us each, 63x). Restrict
# the chooser to sets that either hold both or hold neither.
from concourse.hw_specs import get_activation_tables as _gat


def _gat_shared(arch):
    # Index into this dict is the act_func_set_id consumed by walrus, so
    # entries must stay in place — only strip Exp/Ln from the other sets.
    out = {}
    for name, fns in _gat(arch).items():
        if name != "natural_log_exp_and_others" and ({AF.Exp, AF.Ln} & fns):
            fns = fns - {AF.Exp, AF.Ln}
        out[name] = fns
    return out


bacc.get_activation_tables = _gat_shared


def build_program():
    nc = bacc.Bacc(trn_type="TRN2", name="mha_spmd")
    qT = nc.dram_tensor("qT", [D, L], F32R, kind="ExternalInput")
    kT = nc.dram_tensor("kT", [D, L], F32R, kind="ExternalInput")
    vT = nc.dram_tensor("vT", [D, L], F32R, kind="ExternalInput")
    wq = nc.dram_tensor("wq", [D, HPC * DK], F32R, kind="ExternalInput")
    wk = nc.dram_tensor("wk", [D, HPC * DK], F32R, kind="ExternalInput")
    wv = nc.dram_tensor("wv", [D, HPC * DV], F32R, kind="ExternalInput")
    wo = nc.dram_tensor("wo", [HPC * DV, D], F32R, kind="ExternalInput")
    onesc = nc.dram_tensor("onesc", [128, HPC], F32R, kind="ExternalInput")
    # attn in flat tile order: [head, qb, p, t, q] ; host reassembles
    attn_f = nc.dram_tensor(
        "attn_f", [HPC, NQB, 128, NLT, QBLK], F32R, kind="ExternalOutput"
    )
    out_p = nc.dram_tensor("out_p", [L, D], F32, kind="ExternalOutput")
    # on-device scratch (qh and ctx stream through DRAM to fit SBUF)
    qh_d = nc.dram_tensor("qh_d", [NPAIR, NQB // 2, 128, 2 * QBLK], F32R)
    ctx_d = nc.dram_tensor("ctx_d", [NQB, NPAIR, 2, 64, QBLK], F32R)

    with tile.TileContext(nc) as tc:
        with tc.tile_pool(name="persist", bufs=1) as pp:
            khT = [
                pp.tile([128, L], F32R, tag=f"khT{p}", name=f"khT{p}")
                for p in range(NPAIR)
            ]
            vho = [
                pp.tile([128, HPC * (DV + 1)], F32R, tag=f"vho{t}", name=f"vho{t}")
                for t in range(NLT)
            ]

            # ---------------- stage A: projections ----------------
            with (
                tc.tile_pool(name="wgt", bufs=1) as wp,
                tc.tile_pool(name="stream", bufs=2) as sp,
                tc.tile_pool(name="qstage", bufs=2) as qsp,
                tc.tile_pool(name="apsum", bufs=4, space="PSUM") as appool,
            ):
                # --- q projection -> qh_d (DRAM) ---
                wq_sb = []
                for i in range(NDC):
                    wt = wp.tile([128, HPC * DK], F32R, tag=f"wq{i}", name=f"wq{i}")
                    nc.sync.dma_start(wt[:], wq[i * 128 : (i + 1) * 128, :])
                    wq_sb.append(wt)
                for n in range(NQB // 2):
                    xs = []
                    for kk in range(NDC):
                        xt = sp.tile([128, 512], F32R, tag=f"x{kk}", name=f"x{kk}")
                        nc.sync.dma_start(
                            xt[:],
                            qT[kk * 128 : (kk + 1) * 128, n * 512 : (n + 1) * 512],
                        )
                        xs.append(xt)
                    for m in range(NPAIR):
                        ps = appool.tile([128, 512], F32, tag="projps", name="projps")
                        for kk in range(NDC):
                            nc.tensor.matmul(
                                ps[:],
                                lhsT=wq_sb[kk][:, m * 128 : (m + 1) * 128],
                                rhs=xs[kk][:],
                                start=(kk == 0),
                                stop=(kk == NDC - 1),
                            )
                        qst = qsp.tile([128, 512], F32R, tag="qst", name="qst")
                        nc.vector.tensor_copy(qst[:], ps[:])
                        nc.gpsimd.dma_start(qh_d[m, n], qst[:])

                # --- k projection -> khT (SBUF resident) ---
                wk_sb = []
                for i in range(NDC):
                    wt = wp.tile([128, HPC * DK], F32R, tag=f"wk{i}", name=f"wk{i}")
                    nc.sync.dma_start(wt[:], wk[i * 128 : (i + 1) * 128, :])
                    wk_sb.append(wt)
                for n in range(NQB // 2):
                    xs = []
                    for kk in range(NDC):
                        xt = sp.tile([128, 512], F32R, tag=f"x{kk}", name=f"x{kk}")
                        nc.sync.dma_start(
                            xt[:],
                            kT[kk * 128 : (kk + 1) * 128, n * 512 : (n + 1) * 512],
                        )
                        xs.append(xt)
                    for m in range(NPAIR):
                        ps = appool.tile([128, 512], F32, tag="projps", name="projps")
                        for kk in range(NDC):
                            nc.tensor.matmul(
                                ps[:],
                                lhsT=wk_sb[kk][:, m * 128 : (m + 1) * 128],
                                rhs=xs[kk][:],
                                start=(kk == 0),
                                stop=(kk == NDC - 1),
                            )
                        nc.vector.tensor_copy(
                            khT[m][:, n * 512 : (n + 1) * 512], ps[:]
                        )

                # --- v projection -> vho (SBUF resident, with ones col) ---
                wv_sb = []
                for i in range(NDC):
                    wt = wp.tile([128, HPC * DV], F32R, tag=f"wv{i}", name=f"wv{i}")
                    nc.sync.dma_start(wt[:], wv[i * 128 : (i + 1) * 128, :])
                    wv_sb.append(wt)
                for tg in range(4):
                    xs = []
                    for kk in range(NDC):
                        xt = sp.tile([128, 512], F32R, tag=f"x{kk}", name=f"x{kk}")
                        nc.sync.dma_start(
                            xt[:],
                            vT[kk * 128 : (kk + 1) * 128, tg * 512 : (tg + 1) * 512],
                        )
                        xs.append(xt)
                    for ti in range(4):
                        t = tg * 4 + ti
                        ps = appool.tile([128, 512], F32, tag="projps", name="projps")
                        for kk in range(NDC):
                            nc.tensor.matmul(
                                ps[:],
                                lhsT=xs[kk][:, ti * 128 : (ti + 1) * 128],
                                rhs=wv_sb[kk][:],
                                start=(kk == 0),
                                stop=(kk == NDC - 1),
                            )
                        dst3 = vho[t][:].rearrange("p (h x) -> p h x", x=DV + 1)
                        src3 = ps[:].rearrange("p (h x) -> p h x", x=DV)
                        nc.vector.tensor_copy(dst3[:, :, 0:DV], src3)
                        nc.sync.dma_start(
                            dst3[:, :, DV : DV + 1],
                            onesc[:].rearrange("p (h x) -> p h x", x=1),
                        )

            # ---------------- stage B: attention per (qb, pair) ----------
            with (
                tc.tile_pool(name="wopool", bufs=1) as wop,
                tc.tile_pool(name="epool", bufs=2) as ep,
                tc.tile_pool(name="qread", bufs=2) as qrp,
                tc.tile_pool(name="cstage", bufs=2) as csp,
                tc.tile_pool(name="cread", bufs=1) as crp,
                tc.tile_pool(name="astage", bufs=2) as asp,
                tc.tile_pool(name="small", bufs=2) as smp,
                tc.tile_pool(name="outsb", bufs=1) as osb,
                tc.tile_pool(name="spsum", bufs=2, space="PSUM") as spp,
                tc.tile_pool(name="vpsum", bufs=1, space="PSUM") as vpp,
                tc.tile_pool(name="opsum", bufs=2, space="PSUM") as opp,
            ):
                wo_sb = []
                for i in range(NPAIR):
                    wt = wop.tile([128, D], F32R, tag=f"wo{i}", name=f"wo{i}")
                    nc.sync.dma_start(wt[:], wo[i * 128 : (i + 1) * 128, :])
                    wo_sb.append(wt)

                for qb in range(NQB):
                    for p in range(NPAIR):
                        qhr = qrp.tile([128, QBLK], F32R, tag="qhr", name="qhr")
                        nc.gpsimd.dma_start(
                            qhr[:],
                            qh_d[p, qb // 2][:, (qb % 2) * QBLK : (qb % 2 + 1) * QBLK],
                        )
                        # epair layout per 1024-col group g (t = 2g + t2):
                        #   [A(2g) | A(2g+1) | B(2g) | B(2g+1)]  (256 each)
                        epair = ep.tile(
                            [128, NLT * 2 * QBLK], F32R, tag="epair", name="epair"
                        )
                        pos = [
                            vpp.tile([128, QBLK], F32, tag=f"pv{jj}", name=f"pv{jj}")
                            for jj in range(2)
                        ]
                        for g in range(NLT // 2):
                            sps = spp.tile(
                                [128, 4 * QBLK], F32, tag="score", name="score"
                            )
                            for t2 in range(2):
                                t = 2 * g + t2
                                for jj in range(2):
                                    base = jj * 64
                                    nc.tensor.matmul(
                                        sps[
                                            :,
                                            (jj * 2 + t2) * QBLK : (jj * 2 + t2 + 1)
                                            * QBLK,
                                        ],
                                        lhsT=khT[p][
                                            base : base + 64, t * 128 : (t + 1) * 128
                                        ],
                                        rhs=qhr[base : base + 64, :],
                                        start=True,
                                        stop=True,
                                        tile_position=(base, 0),
                                    )
                            nc.scalar.activation(
                                epair[:, g * 4 * QBLK : (g + 1) * 4 * QBLK],
                                sps[:],
                                AF.Exp,
                                scale=0.125,
                            )
                            for t2 in range(2):
                                t = 2 * g + t2
                                for jj in range(2):
                                    j = p * 2 + jj
                                    nc.tensor.matmul(
                                        pos[jj][0 : DV + 1, :],
                                        lhsT=vho[t][
                                            :, j * (DV + 1) : (j + 1) * (DV + 1)
                                        ],
                                        rhs=epair[
                                            :,
                                            (g * 4 + jj * 2 + t2) * QBLK : (
                                                g * 4 + jj * 2 + t2 + 1
                                            )
                                            * QBLK,
                                        ],
                                        start=(t == 0),
                                        stop=(t == NLT - 1),
                                    )
                        # e5: [p][g 8][jj 2][t2 2][q]
                        e5 = epair[:].rearrange(
                            "p (g jj t2 q) -> p g jj t2 q", jj=2, t2=2, q=QBLK
                        )
                        for jj in range(2):
                            j = p * 2 + jj
                            po = pos[jj]
                            lnd = smp.tile([1, QBLK], F32, tag="lnd", name="lnd")
                            nc.scalar.activation(lnd[:], po[DV : DV + 1, :], AF.Ln)
                            rec = smp.tile([1, QBLK], F32, tag="rec", name="rec")
                            nc.scalar.activation(rec[:], lnd[:], AF.Exp, scale=-1.0)
                            rep = smp.tile([128, QBLK], F32, tag="rep", name="rep")
                            nc.gpsimd.partition_broadcast(rep[:], rec[:])
                            # normalized context -> DRAM (GpSimd: small op)
                            cst = csp.tile([64, QBLK], F32R, tag="cst", name="cst")
                            nc.vector.tensor_mul(cst[:], po[0:DV, :], rep[0:DV, :])
                            nc.gpsimd.dma_start(ctx_d[qb, p, jj], cst[:])
                            # normalize E^T (8 t's per op), stage, DMA out
                            for cg in range(2):
                                ast = asp.tile(
                                    [128, 8 * QBLK], F32R, tag="ast", name="ast"
                                )
                                e3 = e5[:, cg * 4 : (cg + 1) * 4, jj, :, :]
                                a3 = ast[:].rearrange(
                                    "p (g t2 q) -> p g t2 q", t2=2, q=QBLK
                                )
                                r4 = rep[:].rearrange(
                                    "p (g t2 q) -> p g t2 q", g=1, t2=1
                                )
                                r4b, _ = bass.broadcast_tensor_aps(r4, e3)
                                nc.vector.tensor_mul(a3, e3, r4b)
                                nc.sync.dma_start(
                                    attn_f[j, qb, :, cg * 8 : (cg + 1) * 8, :],
                                    a3,
                                )

                    # ---- output projection for this q-block ----
                    ctx_sb = []
                    for kk in range(NPAIR):
                        ct = crp.tile(
                            [128, QBLK], F32R, tag=f"ctr{kk}", name=f"ctr{kk}"
                        )
                        nc.gpsimd.dma_start(
                            ct[:],
                            ctx_d[qb, kk].rearrange("a b c -> (a b) c"),
                        )
                        ctx_sb.append(ct)
                    for mq in range(QBLK // 128):
                        row = slice(qb * QBLK + mq * 128, qb * QBLK + (mq + 1) * 128)
                        ob = osb.tile([128, D], F32, tag="ob", name="ob")
                        for dc in range(2):
                            ps = opp.tile([128, 512], F32, tag="ops", name="ops")
                            for kk in range(NPAIR):
                                nc.tensor.matmul(
                                    ps[:],
                                    lhsT=ctx_sb[kk][:, mq * 128 : (mq + 1) * 128],
                                    rhs=wo_sb[kk][:, dc * 512 : (dc + 1) * 512],
                                    start=(kk == 0),
                                    stop=(kk == NPAIR - 1),
                                )
                            nc.vector.tensor_copy(
                                ob[:, dc * 512 : (dc + 1) * 512], ps[:]
                            )
                        nc.sync.dma_start(out_p[row, :], ob[:])

    nc.finalize()
    return nc


_PROGRAM_CACHE = {}


def _get_program():
    if "nc" not in _PROGRAM_CACHE:
        _PROGRAM_CACHE["nc"] = build_program()
    return _PROGRAM_CACHE["nc"]


def _shard_inputs(q, k, v, Wq, Wk, Wv, Wo):
    WoT = np.ascontiguousarray(Wo.T.astype(np.float32))
    in_maps = []
    for c in range(NCORES):
        b, hf = divmod(c, 2)
        rows = slice(hf * HPC * DK, (hf + 1) * HPC * DK)
        in_maps.append(
            {
                "qT": np.ascontiguousarray(q[b].T.astype(np.float32)),
                "kT": np.ascontiguousarray(k[b].T.astype(np.float32)),
                "vT": np.ascontiguousarray(v[b].T.astype(np.float32)),
                "wq": np.ascontiguousarray(Wq[rows].T.astype(np.float32)),
                "wk": np.ascontiguousarray(Wk[rows].T.astype(np.float32)),
                "wv": np.ascontiguousarray(Wv[rows].T.astype(np.float32)),
                "wo": np.ascontiguousarray(WoT[rows]),
                "onesc": np.ones((128, HPC), np.float32),
            }
        )
    return in_maps


def run(q, k, v, Wq, Wk, Wv, Wo, trace=False, trace_kwargs=None):
    nc = _get_program()
    in_maps = _shard_inputs(q, k, v, Wq, Wk, Wv, Wo)
    res = None
    for attempt in range(3):
        try:
            res = run_bass_kernel_spmd(
                nc,
                in_maps,
                core_ids=list(range(NCORES)),
                trace=trace,
                **(trace_kwargs or {}),
            )
        except Exception:
            if attempt == 2:
                raise
            continue
        # transient NRT failures can hand back the donated zero buffers;
        # a softmax row summing to ~1 proves the kernel really ran
        probe = res.results[0]["attn_f"][0, 0, :, :, 0].sum()
        if 0.5 < probe < 1.5:
            break
    assert res is not None
    out = np.empty((B, L, D), np.float32)
    attn = np.empty((B, H, L, L), np.float32)
    for c in range(NCORES):
        b, hf = divmod(c, 2)
        r = res.results[c]
        if hf == 0:
            out[b] = r["out_p"]
        else:
            out[b] += r["out_p"]
        # attn_f [head, qb, p, t, q]: attn[q_g, k_g] with q_g = qb*QBLK+q,
        # k_g = t*128 + p  ->  transpose to [head, qb, q, t, p]
        af = r["attn_f"]
        attn[b, hf * HPC : (hf + 1) * HPC] = af.transpose(0, 1, 4, 3, 2).reshape(
            HPC, L, L
        )
    return out, attn, res


def kernel(q, k, v, Wq, Wk, Wv, Wo):
    out, attn, _ = run(q, k, v, Wq, Wk, Wv, Wo)
    return out, attn
